# revision 33
# baseline (speedup 1.0000x reference)
"""Trainium2 Bass kernel for nn_Attention_65128884077225.

Math: the reference module broadcasts scores [B,H,S,1] along the softmax
axis, so every softmax row is constant -> attention weights are exactly
uniform (1/S). Hence z = mean_s(v) broadcast over s, and the whole module
collapses to, per batch b:

    c[b] = (mean_s x[b,s,:]) @ Wv @ Wout + (bv @ Wout + bout)
    out[b,s,:] = c[b]                      (constant across s)

where Wv = qkv_w[:, 2E:3E], bv = qkv_b[2E:3E].

Sharding: 8 cores = 4 batches x 2 column-halves. Core c handles batch
b=c//2 and output columns [h*256, (h+1)*256), h=c%2. Each core reads the
full x[b] (needed for the mean), but only its half of the folded weight
matrix, and writes out[b][:, cols] (2 MiB).

Device kernel per core:
  - 16 DMA loads of x row-tiles [128,512], alternating across the two
    HWDGE rings (sync + scalar),
  - serial DVE add-chain accumulates the 16 tiles -> acc [128,512],
  - 4 matmuls vs a ones-vector give column sums xsum^T [128,4],
  - 4-step accumulated matmul xsum @ Wc_half -> row [1,256], + bias,
  - rank-1 matmul broadcasts the row -> [128,256] tile,
  - 16 stores of that tile cover out[b][:, cols] (written as a contiguous
    [2048,256] per-core output, reassembled on host).

Host only: fold Wc = (Wv @ Wout)/S and bc = bv @ Wout + bout (tiny host
GEMM), shard inputs, and concatenate the per-core outputs.
"""

import sys

import numpy as np

if "/opt/trn_rl_repo" not in sys.path and not any(
    p.endswith("trn_rl_repo") for p in sys.path
):
    sys.path.insert(0, "/opt/trn_rl_repo")

import ml_dtypes

import concourse.bacc as bacc
import concourse.mybir as mybir
import concourse.tile as tile
from concourse.bass_utils import run_bass_kernel_spmd

B, S, E = 4, 2048, 512
N_CORES = 8
P = 128
N_XT = S // P          # 16 x-tiles of [128, 512]
EH = E // 2            # 256 output columns per core
NEC = E // P           # 4 contraction chunks for the crow GEMV
BCAST_Q = 4            # SBUF-side replication of the out tile
FP32 = mybir.dt.float32
BF16 = mybir.dt.bfloat16

_CACHE = {}


def build():
    """Build + compile the per-core Bass program (same for every core)."""
    if "nc" in _CACHE:
        return _CACHE["nc"]
    nc = bacc.Bacc(None, target_bir_lowering=False, enable_partition_id=False)
    x_d = nc.dram_tensor("x", [S, E], FP32, kind="ExternalInput")
    w_d = nc.dram_tensor("w", [P, NEC * EH], BF16, kind="ExternalInput")
    b_d = nc.dram_tensor("b", [1, EH], BF16, kind="ExternalInput")
    o_d = nc.dram_tensor("o", [P, (S // P) * EH], FP32, kind="ExternalOutput")

    def ring(i):
        return nc.sync if i % 2 == 0 else nc.scalar

    with tile.TileContext(nc) as tc:
        with (
            tc.tile_pool(name="xp", bufs=N_XT) as xp,
            tc.tile_pool(name="wp", bufs=1) as wp,
            tc.tile_pool(name="sp", bufs=1) as sp,
            tc.tile_pool(name="ps", bufs=1, space="PSUM") as ps,
        ):
            ones_col = sp.tile([P, 1], FP32, tag="ones_col")
            nc.vector.memset(ones_col[:], 1.0)
            ones2 = sp.tile([2, P], BF16, tag="ones2")
            nc.vector.memset(ones2[:], 1.0)

            # PE warm-up (HAM): dummy rank-reductions chained to late tiles
            # keep the PE clocked at 2.4 GHz going into the tail matmuls.
            p_warm = ps.tile([1, E], FP32, tag="warm")

            # ALL inputs on the sync ring: the single-queue FIFO delivers
            # tiles in exact chain order (~725ns apart) with clean
            # single-lane wait thresholds; the scalar ring stays empty for
            # the store. Weights+bias ride at the back (needed ~23us in).
            xts = []
            for t in range(N_XT):
                xt = xp.tile([P, E], FP32, tag="x")
                nc.sync.dma_start(xt[:], x_d[t * P : (t + 1) * P, :])
                xts.append(xt)
                if 8 <= t < 12:
                    nc.tensor.matmul(
                        p_warm[0:1, 0:EH],
                        ones_col[:],
                        xt[:, :EH],
                        start=True,
                        stop=True,
                    )

            wcb = wp.tile([P, NEC * EH], BF16, tag="w")
            nc.sync.dma_start(wcb[:], w_d[:, :])
            # cb row 0 <- crow (copied from PSUM later); row 1 <- bias DMA.
            # The k=2 broadcast matmul then adds the bias for free.
            cb = sp.tile([2, EH], BF16, tag="cb")
            nc.sync.dma_start(cb[1:2, :], b_d[:, :])

            # serial accumulate; starts with a COPY of tile 0 so every chain
            # op has exactly ONE DMA dependency (a single-lane wait -- the
            # two-lane merged event-sem was observed waking ~2.8us late).
            # The final tile's add is split lo/hi so the lo reduction+copy
            # overlaps the hi add.
            # wave A: tiles 0..11 -> partition-reduce + 4 crow matmuls run
            # DURING the read (the PE is otherwise idle); the chain starts
            # with a COPY of tile 0 so every op has one clean DMA dependency
            accA = sp.tile([P, E], FP32, tag="accA")
            nc.vector.tensor_copy(accA[:], xts[0][:])
            for t in range(1, 12):
                nc.vector.tensor_add(accA[:], accA[:], xts[t][:])
            p_redA = ps.tile([P, 4], FP32, tag="redA")
            for c in range(4):
                nc.tensor.matmul(
                    p_redA[:, c : c + 1],
                    accA[:, c * P : (c + 1) * P],
                    ones_col[:],
                    start=True,
                    stop=True,
                )
            xsA = sp.tile([P, 4], BF16, tag="xsA")
            nc.scalar.copy(xsA[:], p_redA[:])
            p_crow = ps.tile([1, EH], FP32, tag="crow")
            for k in range(4):
                nc.tensor.matmul(
                    p_crow[:],
                    xsA[:, k : k + 1],
                    wcb[:, k * EH : (k + 1) * EH],
                    start=(k == 0),
                    stop=False,
                )

            # PE keep-warm gated on the late tiles (wave A's real matmuls
            # cover the window the dummy warm-ups used to)
            nc.tensor.matmul(
                p_warm[0:1, 0:EH], ones_col[:], xts[14][:, :EH], start=True, stop=True
            )
            nc.tensor.matmul(
                p_warm[0:1, 0:EH], ones_col[:], xts[15][:, :EH], start=True, stop=True
            )

            # wave B: tiles 12..15; the final tile's add is split lo/hi so
            # the lo reduction+copy overlaps the hi add
            accB = sp.tile([P, E], FP32, tag="accB")
            nc.vector.tensor_copy(accB[:], xts[12][:])
            nc.vector.tensor_add(accB[:], accB[:], xts[13][:])
            nc.vector.tensor_add(accB[:], accB[:], xts[14][:])
            accB_lo = sp.tile([P, EH], FP32, tag="accB_lo")
            accB_hi = sp.tile([P, EH], FP32, tag="accB_hi")
            nc.vector.tensor_add(accB_lo[:], accB[:, :EH], xts[15][:, :EH])
            nc.vector.tensor_add(accB_hi[:], accB[:, EH:], xts[15][:, EH:])
            p_redB = ps.tile([P, 4], FP32, tag="redB")
            accBs = [accB_lo, accB_lo, accB_hi, accB_hi]
            for c in range(4):
                nc.tensor.matmul(
                    p_redB[:, c : c + 1],
                    accBs[c][:, (c % 2) * P : (c % 2 + 1) * P],
                    ones_col[:],
                    start=True,
                    stop=True,
                )
            xsB_lo = sp.tile([P, 2], BF16, tag="xsB_lo")
            nc.scalar.copy(xsB_lo[:], p_redB[:, 0:2])
            xsB_hi = sp.tile([P, 2], BF16, tag="xsB_hi")
            nc.scalar.copy(xsB_hi[:], p_redB[:, 2:4])
            xBs = [xsB_lo, xsB_lo, xsB_hi, xsB_hi]
            for k in range(4):
                nc.tensor.matmul(
                    p_crow[:],
                    xBs[k][:, k % 2 : k % 2 + 1],
                    wcb[:, k * EH : (k + 1) * EH],
                    start=False,
                    stop=(k == 3),
                )
            # crow PSUM -> cb row 0 on ACT (261ns measured vs 412 on DVE,
            # and ACT is free right after the xsB copies)
            nc.scalar.copy(cb[0:1, :], p_crow[:])

            # broadcast crow+bias across partitions via k=2 matmuls into two
            # PSUM banks so DVE and ACT replicate in parallel afterwards
            p_bc0 = ps.tile([P, EH], FP32, tag="bc0")
            p_bc1 = ps.tile([P, EH], FP32, tag="bc1")
            nc.tensor.matmul(p_bc0[:], ones2[:], cb[:], start=True, stop=True)
            nc.tensor.matmul(p_bc1[:], ones2[:], cb[:], start=True, stop=True)
            bcast = sp.tile([P, BCAST_Q, EH], FP32, tag="bcast")
            nc.vector.tensor_copy(
                bcast[:, 0:2, :], p_bc0[:, None, :].broadcast_to([P, 2, EH])
            )
            nc.scalar.copy(bcast[:, 2, :], p_bc1[:, :])
            nc.scalar.copy(bcast[:, 3, :], p_bc1[:, :])

            # TWO stores on the SAME (scalar) ring: the first gates only on
            # DVE's replicate copy and issues ~1us before ACT's copies
            # finish; the second pipelines behind it in the queue FIFO
            o_t = o_d.rearrange("p (h g q e) -> p h g (q e)", h=2, q=2, e=EH)
            src0 = bcast[:, None, 0:2, :].broadcast_to([P, 4, 2, EH]).rearrange(
                "p g q e -> p g (q e)"
            )
            src1 = bcast[:, None, 2:4, :].broadcast_to([P, 4, 2, EH]).rearrange(
                "p g q e -> p g (q e)"
            )
            nc.scalar.dma_start(o_t[:, 0, :, :], src0)
            nc.scalar.dma_start(o_t[:, 1, :, :], src1)

    nc.compile()
    _CACHE["nc"] = nc
    return nc


def _fold_weights(qkv_w, qkv_b, out_w, out_b):
    wv = np.asarray(qkv_w)[:, 2 * E : 3 * E].astype(np.float64)
    wc = (wv @ np.asarray(out_w).astype(np.float64) / S).astype(np.float32)
    bc = (
        np.asarray(qkv_b)[2 * E : 3 * E].astype(np.float64)
        @ np.asarray(out_w).astype(np.float64)
        + np.asarray(out_b)
    ).astype(np.float32)
    return wc, bc


def _pack_w(wc, h):
    """[128, 4*256] bf16: E-chunk-major packing of this half's Wc columns."""
    cols = slice(h * EH, (h + 1) * EH)
    return np.ascontiguousarray(
        wc[:, cols].reshape(NEC, P, EH).transpose(1, 0, 2).reshape(P, NEC * EH)
    ).astype(ml_dtypes.bfloat16)


def _run(inputs, trace=False, **kwargs):
    nc = build()
    x = np.ascontiguousarray(np.asarray(inputs["x"], dtype=np.float32))
    wc, bc = _fold_weights(
        inputs["qkv_w"], inputs["qkv_b"], inputs["out_w"], inputs["out_b"]
    )
    wpk = [_pack_w(wc, h) for h in range(2)]
    bpk = [
        np.ascontiguousarray(bc[h * EH : (h + 1) * EH].reshape(1, EH)).astype(
            ml_dtypes.bfloat16
        )
        for h in range(2)
    ]
    in_maps = [
        {"x": x[c // 2], "w": wpk[c % 2], "b": bpk[c % 2]} for c in range(N_CORES)
    ]
    res = run_bass_kernel_spmd(
        nc, in_maps, core_ids=list(range(N_CORES)), trace=trace, **kwargs
    )
    out = np.empty((B, S, E), dtype=np.float32)
    for b in range(B):
        for h in range(2):
            o = res.results[2 * b + h]["o"]
            o = o.reshape(P, S // P, EH).transpose(1, 0, 2).reshape(S, EH)
            out[b, :, h * EH : (h + 1) * EH] = o
    return out, res


def kernel(**inputs) -> np.ndarray:
    out, _ = _run(inputs, trace=False)
    return out
